# revision 6
# baseline (speedup 1.0000x reference)
"""Trainium2 Bass kernel for nn_ContextEncoder (segment_reduce).

Computes: out[a, :] = segment_max(pre_seq @ W_in + b_in + pe[pre_timesteps])
with 8192 agents x 20 tokens, D=256, sharded over 8 NeuronCores by agent
(1024 agents / 20480 tokens per core, segments never cross cores).

Device strategy:
- Fold the input FC, bias add and positional-encoding gather into a single
  PE matmul per 128-channel tile: each token's input vector is extended to
  u = [s0, s1, 1, onehot50(t)] (fp16, K=53) against weights
  [W_in; b_in; pe].  u is packed [117, TOK_C/2]: two 53-row token-halves at
  partitions 0 and 64 (wide DMA + PE 64x128 row tiling via tile_position).
- The 20:1 segment max is PSUM-evacuation-bound: only ACT (1.2 GHz) and
  DVE (0.96 GHz) can read PSUM, so the agent blocks are sized to balance
  the two engines exactly: 6 "AD" blocks of 124 agents (ACT copies
  PSUM->fp16 SBUF, DVE runs the 2x-packed TT-max tree; adjacent blocks'
  trees are stacked into shared ops to halve instruction overhead) and
  2 "B" blocks of 140 agents laid out agent-major so one contiguous
  tensor_reduce per PSUM tile collapses k=20 straight into fp32 output.
"""

import sys

for _p in ("/opt/trn_rl_repo", "/root/.axon_site/_ro/trn_rl_repo"):
    if _p not in sys.path:
        sys.path.insert(0, _p)

import numpy as np

import concourse.bacc as bacc
import concourse.bass as bass
import concourse.mybir as mybir
from concourse.tile import TileContext

N_CORES = 8
N_AGENTS = 8192
T = 20
D = 256
N_TOK = N_AGENTS * T            # 163840
TOK_C = N_TOK // N_CORES        # 20480 tokens per core
AG_C = N_AGENTS // N_CORES      # 1024 agents per core
WINDOW = 50
K_U = 3 + WINDOW                # 53: s0, s1, ones, onehot50 (all fp16)

# Non-uniform agent blocks per core: 6 AD blocks + 2 B blocks, in agent
# order, so out_sb column == agent index within the core.
AD_AG = 124
B_AG = 140
N_AD = 6
N_B = 2
assert N_AD * AD_AG + N_B * B_AG == AG_C

AD_TILE = 10 * AD_AG            # 1240 cols per AD psum tile (10 k-slabs)
B_HALF = B_AG // 2              # 70 agents per B psum tile
B_TILE = B_HALF * T             # 1400 cols per B psum tile (agent-major)
PSUM_W = max(AD_TILE, B_TILE)   # 1400 fp32 -> 3 PSUM banks

BLK_N = [AD_AG] * N_AD + [B_AG] * N_B
BLK_OFF = np.cumsum([0] + BLK_N).tolist()   # [0,124,...,620,744,884,1024]

# Slab s holds blocks (2s, 2s+1) at partition rows (0:53, 64:117).
SLAB_PAIRS = [(0, 1), (2, 3), (4, 5), (6, 7)]
SLAB_W = [2 * AD_TILE] * 3 + [2 * B_TILE]

F16 = mybir.dt.float16
F32 = mybir.dt.float32


def _build_pe():
    pos = np.arange(-20, 30, dtype=np.float64)[:, None]
    div = np.exp(np.arange(0, D, 2, dtype=np.float64) * (-np.log(10000.0) / D))
    pe = np.zeros((WINDOW, D), dtype=np.float64)
    pe[:, 0::2] = np.sin(pos * div)
    pe[:, 1::2] = np.cos(pos * div)
    return pe.astype(np.float32)


def _block_token_order(blk):
    """Within-core token index for each of a block's u columns, in order.

    AD blocks (k-major): [tile h][k 0..9][agent]  -> tok = a*T + h*10 + k
    B blocks (agent-major): [tile h][agent 70][k] -> tok = a*T + k
    """
    off, n = BLK_OFF[blk], BLK_N[blk]
    if blk < N_AD:
        h = np.arange(2)[:, None, None]
        k = np.arange(10)[None, :, None]
        a = np.arange(n)[None, None, :]
        tok = (off + a) * T + h * 10 + k
    else:
        half = n // 2
        h = np.arange(2)[:, None, None]
        a = np.arange(half)[None, :, None]
        k = np.arange(T)[None, None, :]
        tok = (off + h * half + a) * T + k
    return tok.reshape(-1)


_PERM_LO = np.concatenate([_block_token_order(lo) for lo, hi in SLAB_PAIRS])
_PERM_HI = np.concatenate([_block_token_order(hi) for lo, hi in SLAB_PAIRS])


def _host_inputs(pre_seq, W_in, b_in, pre_timesteps):
    """Per-core u ([117, TOK_C/2] fp16, slab-packed) + shared wf fp16."""
    pe = _build_pe()
    wf = np.concatenate(
        [
            W_in.astype(np.float16),
            b_in.astype(np.float16)[None],
            pe.astype(np.float16),
        ],
        axis=0,
    )  # [53, 256]

    t = pre_timesteps.astype(np.int64)
    oh = np.zeros((WINDOW, N_TOK), dtype=np.float16)
    oh[t, np.arange(N_TOK)] = 1.0
    ones = np.ones((N_TOK,), dtype=np.float16)
    u_full = np.concatenate(
        [
            pre_seq[:, 0].astype(np.float16)[None],
            pre_seq[:, 1].astype(np.float16)[None],
            ones[None],
            oh,
        ],
        axis=0,
    )  # [53, N_TOK] fp16
    wf2 = np.zeros((117, D), dtype=np.float16)
    wf2[0:53] = wf
    wf2[64:117] = wf
    in_maps = []
    for c in range(N_CORES):
        sl = u_full[:, c * TOK_C : (c + 1) * TOK_C]
        u2 = np.zeros((117, TOK_C // 2), dtype=np.float16)
        u2[0:53] = sl[:, _PERM_LO]
        u2[64:117] = sl[:, _PERM_HI]
        in_maps.append({"u": np.ascontiguousarray(u2), "wf": wf2})
    return in_maps


def _mm_splits(width):
    """<=512-col chunks that never cross a 512-fp32 PSUM bank boundary."""
    splits, off = [], 0
    while off < width:
        n = min(512, width - off)
        splits.append((off, n))
        off += n
    return splits


def _build_nc(reps=1, mm_only=False):
    nc = bacc.Bacc(None)
    u = nc.dram_tensor("u", [117, TOK_C // 2], F16, kind="ExternalInput")
    wf = nc.dram_tensor("wf", [117, D], F16, kind="ExternalInput")
    out = nc.dram_tensor("out", [2, 128, AG_C], F32, kind="ExternalOutput")

    slab_off = np.cumsum([0] + SLAB_W).tolist()

    with TileContext(nc) as tc:
        with (
            tc.tile_pool(name="consts", bufs=1) as consts,
            tc.tile_pool(name="uslabs", bufs=2) as uslabs,
            tc.tile_pool(name="outs", bufs=2) as outs,
            tc.tile_pool(name="psum", bufs=2, space="PSUM") as psum_pool,
            tc.tile_pool(name="hbuf", bufs=3) as hbuf_pool,
            tc.tile_pool(name="s10", bufs=3) as s10_pool,
            tc.tile_pool(name="s5", bufs=3) as s5_pool,
            tc.tile_pool(name="t2", bufs=3) as t2_pool,
            tc.tile_pool(name="hm", bufs=4) as hm_pool,
        ):
            wf_sb = consts.tile([117, D], F16)
            nc.sync.dma_start(out=wf_sb[:], in_=wf[:])

            TTMAX = mybir.AluOpType.max
            X = mybir.AxisListType.X

            for rep in range(reps):
                # 4 wide DMAs; slab s holds blocks 2s (rows 0:53) and 2s+1
                # (rows 64:117); per-tag bufs=2 double-buffers across reps
                slabs = []
                for s in range(4):
                    slab = uslabs.tile(
                        [117, SLAB_W[s]], F16, tag=f"slab{s}", bufs=2,
                        name=f"slab{s}",
                    )
                    nc.sync.dma_start(
                        out=slab[:], in_=u[:, slab_off[s] : slab_off[s + 1]]
                    )
                    slabs.append(slab)

                for m in range(2):
                    out_sb = outs.tile([128, AG_C], F32)
                    if mm_only:
                        nc.gpsimd.memset(out_sb[:], 0.0)
                    pend = None  # staged hb awaiting the pair's second block
                    for blk in range(8):
                        s = blk // 2
                        row0 = 64 if blk % 2 else 0
                        rows = slice(row0, row0 + K_U)
                        tpos = (row0, 0)
                        lhsT = wf_sb[rows, m * 128 : (m + 1) * 128]
                        tw = AD_TILE if blk < N_AD else B_TILE
                        pts = []
                        for h in range(2):
                            pt = psum_pool.tile([128, PSUM_W], F32)
                            for off, n in _mm_splits(tw):
                                o = h * tw + off
                                nc.tensor.matmul(
                                    pt[:, off : off + n],
                                    lhsT,
                                    slabs[s][rows, o : o + n],
                                    tile_position=tpos,
                                )
                            pts.append(pt)
                        if mm_only:
                            continue
                        if blk >= N_AD:
                            # B block: agent-major tiles; one contiguous
                            # tensor_reduce per tile straight to fp32 out
                            for h in range(2):
                                o = BLK_OFF[blk] + h * B_HALF
                                nc.vector.tensor_reduce(
                                    out_sb[:, o : o + B_HALF],
                                    pts[h][:, 0:B_TILE].rearrange(
                                        "p (a k) -> p a k", k=T
                                    ),
                                    axis=X,
                                    op=TTMAX,
                                )
                            continue
                        # AD block: ACT evacuates both tiles into this
                        # pair's interleaved quarters of hb
                        if pend is None:
                            hb = hbuf_pool.tile([128, 4 * AD_TILE], F16)
                            nc.scalar.copy(
                                hb[:, 0:AD_TILE], pts[0][:, 0:AD_TILE]
                            )
                            nc.scalar.copy(
                                hb[:, 2 * AD_TILE : 3 * AD_TILE],
                                pts[1][:, 0:AD_TILE],
                            )
                            pend = hb
                            continue
                        hb = pend
                        pend = None
                        nc.scalar.copy(
                            hb[:, AD_TILE : 2 * AD_TILE], pts[0][:, 0:AD_TILE]
                        )
                        nc.scalar.copy(
                            hb[:, 3 * AD_TILE : 4 * AD_TILE],
                            pts[1][:, 0:AD_TILE],
                        )
                        # Stacked 20->1 max tree for blocks (blk-1, blk).
                        # hb = [A-t0 | B-t0 | A-t1 | B-t1], each 1240 cols
                        # (10 k-slabs of 124 agents); A/B lanes stay
                        # separate via [p, u, w] views (u-stride = per-op
                        # buffer half).
                        W2 = 2 * AD_TILE
                        s10 = s10_pool.tile([128, W2], F16)
                        nc.vector.tensor_tensor(
                            s10[:], hb[:, 0:W2], hb[:, W2 : 2 * W2], op=TTMAX
                        )
                        w5 = 5 * AD_AG
                        v10 = s10[:].rearrange("p (u w) -> p u w", u=2)
                        s5 = s5_pool.tile([128, 2 * w5], F16)
                        v5 = s5[:].rearrange("p (u w) -> p u w", u=2)
                        nc.vector.tensor_tensor(
                            v5, v10[:, :, 0:w5], v10[:, :, w5 : 2 * w5],
                            op=TTMAX,
                        )
                        wq = 2 * AD_AG
                        t2 = t2_pool.tile([128, 2 * wq], F16)
                        v2 = t2[:].rearrange("p (u w) -> p u w", u=2)
                        nc.vector.tensor_tensor(
                            v2, v5[:, :, 0:wq], v5[:, :, wq : 2 * wq],
                            op=TTMAX,
                        )
                        t1 = hm_pool.tile([128, 2 * AD_AG], F16)
                        v1 = t1[:].rearrange("p (u w) -> p u w", u=2)
                        nc.vector.tensor_tensor(
                            v1, v2[:, :, 0:AD_AG], v2[:, :, AD_AG : 2 * AD_AG],
                            op=TTMAX,
                        )
                        # Final: max with each unit's 5th k-slab, written to
                        # the pair's contiguous 248-agent span of out_sb.
                        o0 = BLK_OFF[blk - 1]
                        ov = out_sb[:, o0 : o0 + 2 * AD_AG].rearrange(
                            "p (u w) -> p u w", u=2
                        )
                        nc.vector.tensor_tensor(
                            ov, v1[:], v5[:, :, 4 * AD_AG : 5 * AD_AG],
                            op=TTMAX,
                        )
                    nc.sync.dma_start(out=out[m], in_=out_sb[:])

    nc.finalize()
    return nc


_RUNNER = None


def _make_runner():
    """Compile once; return callable(list of per-core input dicts) -> results."""
    import jax
    from jax.sharding import Mesh, PartitionSpec
    from jax.experimental.shard_map import shard_map
    from concourse import bass2jax
    from concourse.bass2jax import _bass_exec_p, partition_id_tensor

    nc = _build_nc()
    bass2jax.install_neuronx_cc_hook()

    partition_name = nc.partition_id_tensor.name if nc.partition_id_tensor else None
    in_names, out_names, out_avals, zero_outs = [], [], [], []
    for alloc in nc.m.functions[0].allocations:
        if not isinstance(alloc, mybir.MemoryLocationSet):
            continue
        name = alloc.memorylocations[0].name
        if alloc.kind == "ExternalInput":
            if name != partition_name:
                in_names.append(name)
        elif alloc.kind == "ExternalOutput":
            out_names.append(name)
            shape = tuple(alloc.tensor_shape)
            dtype = mybir.dt.np(alloc.dtype)
            out_avals.append(jax.core.ShapedArray(shape, dtype))
            zero_outs.append(np.zeros(shape, dtype))
    n_params = len(in_names)
    n_outs = len(out_avals)
    all_in_names = in_names + out_names
    if partition_name is not None:
        all_in_names.append(partition_name)

    def _body(*args):
        operands = list(args)
        if partition_name is not None:
            operands.append(partition_id_tensor())
        outs = _bass_exec_p.bind(
            *operands,
            out_avals=tuple(out_avals),
            in_names=tuple(all_in_names),
            out_names=tuple(out_names),
            lowering_input_output_aliases=(),
            sim_require_finite=True,
            sim_require_nnan=True,
            nc=nc,
        )
        return tuple(outs)

    devices = jax.devices()[:N_CORES]
    mesh = Mesh(np.asarray(devices), ("core",))
    in_specs = (PartitionSpec("core"),) * (n_params + n_outs)
    out_specs = (PartitionSpec("core"),) * n_outs
    donate = tuple(range(n_params, n_params + n_outs))
    sharded = jax.jit(
        shard_map(_body, mesh=mesh, in_specs=in_specs, out_specs=out_specs,
                  check_rep=False),
        donate_argnums=donate,
        keep_unused=True,
    )

    def run(in_maps):
        per_core = [[np.asarray(m[name]) for name in in_names] for m in in_maps]
        concat_in = [
            np.concatenate([per_core[c][i] for c in range(N_CORES)], axis=0)
            for i in range(n_params)
        ]
        concat_zeros = [
            np.zeros((N_CORES * z.shape[0], *z.shape[1:]), z.dtype) for z in zero_outs
        ]
        out_arrs = sharded(*concat_in, *concat_zeros)
        return [
            {
                name: np.asarray(out_arrs[i]).reshape(N_CORES, *out_avals[i].shape)[c]
                for i, name in enumerate(out_names)
            }
            for c in range(N_CORES)
        ]

    return run


def _get_runner():
    global _RUNNER
    if _RUNNER is None:
        _RUNNER = _make_runner()
    return _RUNNER


def _make_timed(nc, in_maps_fn):
    """Zero-host-transfer callable for steady-state timing (no donation)."""
    import jax
    from jax.sharding import Mesh, PartitionSpec, NamedSharding
    from jax.experimental.shard_map import shard_map
    from concourse import bass2jax
    from concourse.bass2jax import _bass_exec_p, partition_id_tensor

    bass2jax.install_neuronx_cc_hook()
    partition_name = nc.partition_id_tensor.name if nc.partition_id_tensor else None
    in_names, out_names, out_avals = [], [], []
    for alloc in nc.m.functions[0].allocations:
        if not isinstance(alloc, mybir.MemoryLocationSet):
            continue
        name = alloc.memorylocations[0].name
        if alloc.kind == "ExternalInput":
            if name != partition_name:
                in_names.append(name)
        elif alloc.kind == "ExternalOutput":
            out_names.append(name)
            out_avals.append(
                jax.core.ShapedArray(tuple(alloc.tensor_shape), mybir.dt.np(alloc.dtype))
            )
    n_params = len(in_names)
    all_in_names = in_names + out_names + ([partition_name] if partition_name else [])

    def _body(*args):
        operands = list(args)
        if partition_name is not None:
            operands.append(partition_id_tensor())
        outs = _bass_exec_p.bind(
            *operands,
            out_avals=tuple(out_avals),
            in_names=tuple(all_in_names),
            out_names=tuple(out_names),
            lowering_input_output_aliases=(),
            sim_require_finite=True,
            sim_require_nnan=True,
            nc=nc,
        )
        return tuple(outs)

    devices = jax.devices()[:N_CORES]
    mesh = Mesh(np.asarray(devices), ("core",))
    nout = len(out_names)
    sharded = jax.jit(
        shard_map(
            _body,
            mesh=mesh,
            in_specs=(PartitionSpec("core"),) * (n_params + nout),
            out_specs=(PartitionSpec("core"),) * nout,
            check_rep=False,
        ),
        keep_unused=True,
    )
    sh = NamedSharding(mesh, PartitionSpec("core"))
    in_maps = in_maps_fn()
    per_core = [[np.asarray(m[name]) for name in in_names] for m in in_maps]
    dev_in = [
        jax.device_put(
            np.concatenate([per_core[c][i] for c in range(N_CORES)], axis=0), sh
        )
        for i in range(n_params)
    ]
    dev_zero = [
        jax.device_put(np.zeros((N_CORES * a.shape[0], *a.shape[1:]), a.dtype), sh)
        for a in out_avals
    ]

    def run():
        return sharded(*dev_in, *dev_zero)

    return run


def _get_timed_callable(inputs, reps=1, mm_only=False):
    nc = _build_nc(reps=reps, mm_only=mm_only)
    return _make_timed(
        nc,
        lambda: _host_inputs(
            inputs["pre_seq"], inputs["W_in"], inputs["b_in"], inputs["pre_timesteps"]
        ),
    )


def kernel(pre_seq, W_in, b_in, pre_timesteps, pre_agents, n_agents):
    run = _get_runner()
    in_maps = _host_inputs(pre_seq, W_in, b_in, pre_timesteps)
    results = run(in_maps)
    out = np.empty((N_AGENTS, D), dtype=np.float32)
    for c in range(N_CORES):
        o = results[c]["out"]  # [2, 128, AG_C]
        out[c * AG_C : (c + 1) * AG_C] = o.transpose(2, 0, 1).reshape(AG_C, D)
    return out


# revision 7
# speedup vs baseline: 1.1810x; 1.1810x over previous
"""Trainium2 Bass kernel for nn_ContextEncoder (segment_reduce).

Computes: out[a, :] = segment_max(pre_seq @ W_in + b_in + pe[pre_timesteps])
with 8192 agents x 20 tokens, D=256, sharded over 8 NeuronCores by agent
(1024 agents / 20480 tokens per core, segments never cross cores).

Device strategy:
- Fold the input FC, bias add and positional-encoding gather into a single
  PE matmul per 128-channel tile: each token's input vector is extended to
  u = [s0, s1, 1, onehot50(t)] (fp16, K=53) against weights
  [W_in; b_in; pe].  u is packed [117, TOK_C/2]: two 53-row token-halves at
  partitions 0 and 64 (wide DMA + PE 64x128 row tiling via tile_position).
- The 20:1 segment max is PSUM-evacuation-bound: only ACT (1.2 GHz) and
  DVE (0.96 GHz) can read PSUM, so the agent blocks are sized to balance
  the two engines exactly: 6 "AD" blocks of 124 agents (ACT copies
  PSUM->fp16 SBUF, DVE runs the 2x-packed TT-max tree; adjacent blocks'
  trees are stacked into shared ops to halve instruction overhead) and
  2 "B" blocks of 140 agents laid out agent-major so one contiguous
  tensor_reduce per PSUM tile collapses k=20 straight into fp32 output.
"""

import sys

for _p in ("/opt/trn_rl_repo", "/root/.axon_site/_ro/trn_rl_repo"):
    if _p not in sys.path:
        sys.path.insert(0, _p)

import numpy as np

import concourse.bacc as bacc
import concourse.bass as bass
import concourse.mybir as mybir
from concourse.tile import TileContext

N_CORES = 8
N_AGENTS = 8192
T = 20
D = 256
N_TOK = N_AGENTS * T            # 163840
TOK_C = N_TOK // N_CORES        # 20480 tokens per core
AG_C = N_AGENTS // N_CORES      # 1024 agents per core
WINDOW = 50
K_U = 3 + WINDOW                # 53: s0, s1, ones, onehot50 (all fp16)

# Non-uniform agent blocks per core: 6 AD blocks + 2 B blocks, in agent
# order, so out_sb column == agent index within the core.
AD_AG = 124
B_AG = 140
N_AD = 6
N_B = 2
assert N_AD * AD_AG + N_B * B_AG == AG_C

AD_TILE = 10 * AD_AG            # 1240 cols per AD psum tile (10 k-slabs)
B_HALF = B_AG // 2              # 70 agents per B psum tile
B_TILE = B_HALF * T             # 1400 cols per B psum tile (agent-major)
PSUM_W = max(AD_TILE, B_TILE)   # 1400 fp32 -> 3 PSUM banks

BLK_N = [AD_AG] * N_AD + [B_AG] * N_B
BLK_OFF = np.cumsum([0] + BLK_N).tolist()   # [0,124,...,620,744,884,1024]

# Slab s holds blocks (2s, 2s+1) at partition rows (0:53, 64:117).
SLAB_PAIRS = [(0, 1), (2, 3), (4, 5), (6, 7)]
SLAB_W = [2 * AD_TILE] * 3 + [2 * B_TILE]

F16 = mybir.dt.float16
F32 = mybir.dt.float32


def _build_pe():
    pos = np.arange(-20, 30, dtype=np.float64)[:, None]
    div = np.exp(np.arange(0, D, 2, dtype=np.float64) * (-np.log(10000.0) / D))
    pe = np.zeros((WINDOW, D), dtype=np.float64)
    pe[:, 0::2] = np.sin(pos * div)
    pe[:, 1::2] = np.cos(pos * div)
    return pe.astype(np.float32)


def _block_token_order(blk):
    """Within-core token index for each of a block's u columns, in order.

    AD blocks (k-major): [tile h][k 0..9][agent]  -> tok = a*T + h*10 + k
    B blocks (agent-major): [tile h][agent 70][k] -> tok = a*T + k
    """
    off, n = BLK_OFF[blk], BLK_N[blk]
    if blk < N_AD:
        h = np.arange(2)[:, None, None]
        k = np.arange(10)[None, :, None]
        a = np.arange(n)[None, None, :]
        tok = (off + a) * T + h * 10 + k
    else:
        half = n // 2
        h = np.arange(2)[:, None, None]
        a = np.arange(half)[None, :, None]
        k = np.arange(T)[None, None, :]
        tok = (off + h * half + a) * T + k
    return tok.reshape(-1)


_PERM_LO = np.concatenate([_block_token_order(lo) for lo, hi in SLAB_PAIRS])
_PERM_HI = np.concatenate([_block_token_order(hi) for lo, hi in SLAB_PAIRS])


def _host_inputs(pre_seq, W_in, b_in, pre_timesteps):
    """Per-core u ([117, TOK_C/2] fp16, slab-packed) + shared wf fp16."""
    pe = _build_pe()
    wf = np.concatenate(
        [
            W_in.astype(np.float16),
            b_in.astype(np.float16)[None],
            pe.astype(np.float16),
        ],
        axis=0,
    )  # [53, 256]

    t = pre_timesteps.astype(np.int64)
    oh = np.zeros((WINDOW, N_TOK), dtype=np.float16)
    oh[t, np.arange(N_TOK)] = 1.0
    ones = np.ones((N_TOK,), dtype=np.float16)
    u_full = np.concatenate(
        [
            pre_seq[:, 0].astype(np.float16)[None],
            pre_seq[:, 1].astype(np.float16)[None],
            ones[None],
            oh,
        ],
        axis=0,
    )  # [53, N_TOK] fp16
    wf2 = np.zeros((117, D), dtype=np.float16)
    wf2[0:53] = wf
    wf2[64:117] = wf
    in_maps = []
    for c in range(N_CORES):
        sl = u_full[:, c * TOK_C : (c + 1) * TOK_C]
        u2 = np.zeros((117, TOK_C // 2), dtype=np.float16)
        u2[0:53] = sl[:, _PERM_LO]
        u2[64:117] = sl[:, _PERM_HI]
        in_maps.append({"u": np.ascontiguousarray(u2), "wf": wf2})
    return in_maps


def _mm_splits(width):
    """<=512-col chunks that never cross a 512-fp32 PSUM bank boundary."""
    splits, off = [], 0
    while off < width:
        n = min(512, width - off)
        splits.append((off, n))
        off += n
    return splits


def _build_nc(reps=1, mm_only=False):
    nc = bacc.Bacc(None)
    u = nc.dram_tensor("u", [117, TOK_C // 2], F16, kind="ExternalInput")
    wf = nc.dram_tensor("wf", [117, D], F16, kind="ExternalInput")
    out = nc.dram_tensor("out", [2, 128, AG_C], F32, kind="ExternalOutput")

    slab_off = np.cumsum([0] + SLAB_W).tolist()

    with TileContext(nc) as tc:
        with (
            tc.tile_pool(name="consts", bufs=1) as consts,
            tc.tile_pool(name="uslabs", bufs=2) as uslabs,
            tc.tile_pool(name="outs", bufs=2) as outs,
            tc.tile_pool(name="psum", bufs=2, space="PSUM") as psum_pool,
            tc.tile_pool(name="hbuf", bufs=3) as hbuf_pool,
            tc.tile_pool(name="s10", bufs=3) as s10_pool,
            tc.tile_pool(name="s5", bufs=3) as s5_pool,
            tc.tile_pool(name="t2", bufs=3) as t2_pool,
            tc.tile_pool(name="hm", bufs=4) as hm_pool,
        ):
            wf_sb = consts.tile([117, D], F16)
            nc.sync.dma_start(out=wf_sb[:], in_=wf[:])

            TTMAX = mybir.AluOpType.max
            X = mybir.AxisListType.X

            for rep in range(reps):
                # 4 wide DMAs; slab s holds blocks 2s (rows 0:53) and 2s+1
                # (rows 64:117); per-tag bufs=2 double-buffers across reps
                slabs = []
                for s in range(4):
                    slab = uslabs.tile(
                        [117, SLAB_W[s]], F16, tag=f"slab{s}", bufs=2,
                        name=f"slab{s}",
                    )
                    nc.sync.dma_start(
                        out=slab[:], in_=u[:, slab_off[s] : slab_off[s + 1]]
                    )
                    slabs.append(slab)

                for m in range(2):
                    out_sb = outs.tile([128, AG_C], F32)
                    if mm_only:
                        nc.gpsimd.memset(out_sb[:], 0.0)
                    pend = None  # staged hb awaiting the pair's second block
                    # B blocks (DVE-only) interleaved between AD pairs
                    # (ACT-heavy) so both engines stay busy throughout.
                    for blk in (0, 1, 6, 2, 3, 7, 4, 5):
                        s = blk // 2
                        row0 = 64 if blk % 2 else 0
                        rows = slice(row0, row0 + K_U)
                        tpos = (row0, 0)
                        lhsT = wf_sb[rows, m * 128 : (m + 1) * 128]
                        tw = AD_TILE if blk < N_AD else B_TILE
                        pts = []
                        for h in range(2):
                            pt = psum_pool.tile([128, PSUM_W], F32)
                            for off, n in _mm_splits(tw):
                                o = h * tw + off
                                nc.tensor.matmul(
                                    pt[:, off : off + n],
                                    lhsT,
                                    slabs[s][rows, o : o + n],
                                    tile_position=tpos,
                                )
                            pts.append(pt)
                        if mm_only:
                            continue
                        if blk >= N_AD:
                            # B block: agent-major tiles; one contiguous
                            # tensor_reduce per tile straight to fp32 out
                            for h in range(2):
                                o = BLK_OFF[blk] + h * B_HALF
                                nc.vector.tensor_reduce(
                                    out_sb[:, o : o + B_HALF],
                                    pts[h][:, 0:B_TILE].rearrange(
                                        "p (a k) -> p a k", k=T
                                    ),
                                    axis=X,
                                    op=TTMAX,
                                )
                            continue
                        # AD block: ACT evacuates both tiles into this
                        # pair's interleaved quarters of hb
                        if pend is None:
                            hb = hbuf_pool.tile([128, 4 * AD_TILE], F16)
                            nc.scalar.copy(
                                hb[:, 0:AD_TILE], pts[0][:, 0:AD_TILE]
                            )
                            nc.scalar.copy(
                                hb[:, 2 * AD_TILE : 3 * AD_TILE],
                                pts[1][:, 0:AD_TILE],
                            )
                            pend = hb
                            continue
                        hb = pend
                        pend = None
                        nc.scalar.copy(
                            hb[:, AD_TILE : 2 * AD_TILE], pts[0][:, 0:AD_TILE]
                        )
                        nc.scalar.copy(
                            hb[:, 3 * AD_TILE : 4 * AD_TILE],
                            pts[1][:, 0:AD_TILE],
                        )
                        # Stacked 20->1 max tree for blocks (blk-1, blk).
                        # hb = [A-t0 | B-t0 | A-t1 | B-t1], each 1240 cols
                        # (10 k-slabs of 124 agents); A/B lanes stay
                        # separate via [p, u, w] views (u-stride = per-op
                        # buffer half).
                        W2 = 2 * AD_TILE
                        s10 = s10_pool.tile([128, W2], F16)
                        nc.vector.tensor_tensor(
                            s10[:], hb[:, 0:W2], hb[:, W2 : 2 * W2], op=TTMAX
                        )
                        w5 = 5 * AD_AG
                        v10 = s10[:].rearrange("p (u w) -> p u w", u=2)
                        s5 = s5_pool.tile([128, 2 * w5], F16)
                        v5 = s5[:].rearrange("p (u w) -> p u w", u=2)
                        nc.vector.tensor_tensor(
                            v5, v10[:, :, 0:w5], v10[:, :, w5 : 2 * w5],
                            op=TTMAX,
                        )
                        wq = 2 * AD_AG
                        t2 = t2_pool.tile([128, 2 * wq], F16)
                        v2 = t2[:].rearrange("p (u w) -> p u w", u=2)
                        nc.vector.tensor_tensor(
                            v2, v5[:, :, 0:wq], v5[:, :, wq : 2 * wq],
                            op=TTMAX,
                        )
                        t1 = hm_pool.tile([128, 2 * AD_AG], F16)
                        v1 = t1[:].rearrange("p (u w) -> p u w", u=2)
                        nc.vector.tensor_tensor(
                            v1, v2[:, :, 0:AD_AG], v2[:, :, AD_AG : 2 * AD_AG],
                            op=TTMAX,
                        )
                        # Final: max with each unit's 5th k-slab, written to
                        # the pair's contiguous 248-agent span of out_sb.
                        o0 = BLK_OFF[blk - 1]
                        ov = out_sb[:, o0 : o0 + 2 * AD_AG].rearrange(
                            "p (u w) -> p u w", u=2
                        )
                        nc.vector.tensor_tensor(
                            ov, v1[:], v5[:, :, 4 * AD_AG : 5 * AD_AG],
                            op=TTMAX,
                        )
                    nc.sync.dma_start(out=out[m], in_=out_sb[:])

    nc.finalize()
    return nc


_RUNNER = None


def _make_runner():
    """Compile once; return callable(list of per-core input dicts) -> results."""
    import jax
    from jax.sharding import Mesh, PartitionSpec
    from jax.experimental.shard_map import shard_map
    from concourse import bass2jax
    from concourse.bass2jax import _bass_exec_p, partition_id_tensor

    nc = _build_nc()
    bass2jax.install_neuronx_cc_hook()

    partition_name = nc.partition_id_tensor.name if nc.partition_id_tensor else None
    in_names, out_names, out_avals, zero_outs = [], [], [], []
    for alloc in nc.m.functions[0].allocations:
        if not isinstance(alloc, mybir.MemoryLocationSet):
            continue
        name = alloc.memorylocations[0].name
        if alloc.kind == "ExternalInput":
            if name != partition_name:
                in_names.append(name)
        elif alloc.kind == "ExternalOutput":
            out_names.append(name)
            shape = tuple(alloc.tensor_shape)
            dtype = mybir.dt.np(alloc.dtype)
            out_avals.append(jax.core.ShapedArray(shape, dtype))
            zero_outs.append(np.zeros(shape, dtype))
    n_params = len(in_names)
    n_outs = len(out_avals)
    all_in_names = in_names + out_names
    if partition_name is not None:
        all_in_names.append(partition_name)

    def _body(*args):
        operands = list(args)
        if partition_name is not None:
            operands.append(partition_id_tensor())
        outs = _bass_exec_p.bind(
            *operands,
            out_avals=tuple(out_avals),
            in_names=tuple(all_in_names),
            out_names=tuple(out_names),
            lowering_input_output_aliases=(),
            sim_require_finite=True,
            sim_require_nnan=True,
            nc=nc,
        )
        return tuple(outs)

    devices = jax.devices()[:N_CORES]
    mesh = Mesh(np.asarray(devices), ("core",))
    in_specs = (PartitionSpec("core"),) * (n_params + n_outs)
    out_specs = (PartitionSpec("core"),) * n_outs
    donate = tuple(range(n_params, n_params + n_outs))
    sharded = jax.jit(
        shard_map(_body, mesh=mesh, in_specs=in_specs, out_specs=out_specs,
                  check_rep=False),
        donate_argnums=donate,
        keep_unused=True,
    )

    def run(in_maps):
        per_core = [[np.asarray(m[name]) for name in in_names] for m in in_maps]
        concat_in = [
            np.concatenate([per_core[c][i] for c in range(N_CORES)], axis=0)
            for i in range(n_params)
        ]
        concat_zeros = [
            np.zeros((N_CORES * z.shape[0], *z.shape[1:]), z.dtype) for z in zero_outs
        ]
        out_arrs = sharded(*concat_in, *concat_zeros)
        return [
            {
                name: np.asarray(out_arrs[i]).reshape(N_CORES, *out_avals[i].shape)[c]
                for i, name in enumerate(out_names)
            }
            for c in range(N_CORES)
        ]

    return run


def _get_runner():
    global _RUNNER
    if _RUNNER is None:
        _RUNNER = _make_runner()
    return _RUNNER


def _make_timed(nc, in_maps_fn):
    """Zero-host-transfer callable for steady-state timing (no donation)."""
    import jax
    from jax.sharding import Mesh, PartitionSpec, NamedSharding
    from jax.experimental.shard_map import shard_map
    from concourse import bass2jax
    from concourse.bass2jax import _bass_exec_p, partition_id_tensor

    bass2jax.install_neuronx_cc_hook()
    partition_name = nc.partition_id_tensor.name if nc.partition_id_tensor else None
    in_names, out_names, out_avals = [], [], []
    for alloc in nc.m.functions[0].allocations:
        if not isinstance(alloc, mybir.MemoryLocationSet):
            continue
        name = alloc.memorylocations[0].name
        if alloc.kind == "ExternalInput":
            if name != partition_name:
                in_names.append(name)
        elif alloc.kind == "ExternalOutput":
            out_names.append(name)
            out_avals.append(
                jax.core.ShapedArray(tuple(alloc.tensor_shape), mybir.dt.np(alloc.dtype))
            )
    n_params = len(in_names)
    all_in_names = in_names + out_names + ([partition_name] if partition_name else [])

    def _body(*args):
        operands = list(args)
        if partition_name is not None:
            operands.append(partition_id_tensor())
        outs = _bass_exec_p.bind(
            *operands,
            out_avals=tuple(out_avals),
            in_names=tuple(all_in_names),
            out_names=tuple(out_names),
            lowering_input_output_aliases=(),
            sim_require_finite=True,
            sim_require_nnan=True,
            nc=nc,
        )
        return tuple(outs)

    devices = jax.devices()[:N_CORES]
    mesh = Mesh(np.asarray(devices), ("core",))
    nout = len(out_names)
    sharded = jax.jit(
        shard_map(
            _body,
            mesh=mesh,
            in_specs=(PartitionSpec("core"),) * (n_params + nout),
            out_specs=(PartitionSpec("core"),) * nout,
            check_rep=False,
        ),
        keep_unused=True,
    )
    sh = NamedSharding(mesh, PartitionSpec("core"))
    in_maps = in_maps_fn()
    per_core = [[np.asarray(m[name]) for name in in_names] for m in in_maps]
    dev_in = [
        jax.device_put(
            np.concatenate([per_core[c][i] for c in range(N_CORES)], axis=0), sh
        )
        for i in range(n_params)
    ]
    dev_zero = [
        jax.device_put(np.zeros((N_CORES * a.shape[0], *a.shape[1:]), a.dtype), sh)
        for a in out_avals
    ]

    def run():
        return sharded(*dev_in, *dev_zero)

    return run


def _get_timed_callable(inputs, reps=1, mm_only=False):
    nc = _build_nc(reps=reps, mm_only=mm_only)
    return _make_timed(
        nc,
        lambda: _host_inputs(
            inputs["pre_seq"], inputs["W_in"], inputs["b_in"], inputs["pre_timesteps"]
        ),
    )


def kernel(pre_seq, W_in, b_in, pre_timesteps, pre_agents, n_agents):
    run = _get_runner()
    in_maps = _host_inputs(pre_seq, W_in, b_in, pre_timesteps)
    results = run(in_maps)
    out = np.empty((N_AGENTS, D), dtype=np.float32)
    for c in range(N_CORES):
        o = results[c]["out"]  # [2, 128, AG_C]
        out[c * AG_C : (c + 1) * AG_C] = o.transpose(2, 0, 1).reshape(AG_C, D)
    return out


# revision 11
# speedup vs baseline: 1.4563x; 1.2331x over previous
"""Trainium2 Bass kernel for nn_ContextEncoder (segment_reduce).

Computes: out[a, :] = segment_max(pre_seq @ W_in + b_in + pe[pre_timesteps])
with 8192 agents x 20 tokens, D=256, sharded over 8 NeuronCores by agent
(1024 agents / 20480 tokens per core, segments never cross cores).

Device strategy:
- Fold the input FC, bias add and positional-encoding gather into a single
  PE matmul per 128-channel tile: each token's input vector is extended to
  u = [s0, s1, 1, onehot50(t)] (fp16, K=53) against weights
  [W_in; b_in; pe].  u is packed [117, TOK_C/2]: two 53-row token-halves at
  partitions 0 and 64 (wide DMA + PE 64x128 row tiling via tile_position).
- The 20:1 segment max is PSUM-evacuation-bound: only ACT (1.2 GHz) and
  DVE (0.96 GHz) can read PSUM, so the agent blocks are sized to balance
  the two engines exactly: 6 "AD" blocks of 122 agents (ACT copies
  PSUM->fp16 SBUF, DVE runs the 2x-packed TT-max tree; three adjacent
  blocks' trees are stacked into shared ops to cut instruction overhead)
  and 2 "B" blocks of 146 agents laid out agent-major so one contiguous
  tensor_reduce per PSUM tile collapses k=20 straight into fp32 output.
"""

import sys

for _p in ("/opt/trn_rl_repo", "/root/.axon_site/_ro/trn_rl_repo"):
    if _p not in sys.path:
        sys.path.insert(0, _p)

import numpy as np

import concourse.bacc as bacc
import concourse.bass as bass
import concourse.mybir as mybir
from concourse.tile import TileContext

N_CORES = 8
N_AGENTS = 8192
T = 20
D = 256
N_TOK = N_AGENTS * T            # 163840
TOK_C = N_TOK // N_CORES        # 20480 tokens per core
AG_C = N_AGENTS // N_CORES      # 1024 agents per core
WINDOW = 50
K_U = 3 + WINDOW                # 53: s0, s1, ones, onehot50 (all fp16)

# Non-uniform agent blocks per core: 6 AD blocks + 2 B blocks, in agent
# order, so out_sb column == agent index within the core.
AD_AG = 122
B_AG = 146
N_AD = 6
N_B = 2
assert N_AD * AD_AG + N_B * B_AG == AG_C

AD_TILE = 10 * AD_AG            # 1240 cols per AD psum tile (10 k-slabs)
B_HALF = B_AG // 2              # 70 agents per B psum tile
B_TILE = B_HALF * T             # 1400 cols per B psum tile (agent-major)
PSUM_W = max(AD_TILE, B_TILE)   # 1400 fp32 -> 3 PSUM banks

BLK_N = [AD_AG] * N_AD + [B_AG] * N_B
BLK_OFF = np.cumsum([0] + BLK_N).tolist()   # [0,124,...,620,744,884,1024]

# Slab s holds blocks (2s, 2s+1) at partition rows (0:53, 64:117).
SLAB_PAIRS = [(0, 1), (2, 3), (4, 5), (6, 7)]
SLAB_W = [2 * AD_TILE] * 3 + [2 * B_TILE]

F16 = mybir.dt.float16
F32 = mybir.dt.float32


def _build_pe():
    pos = np.arange(-20, 30, dtype=np.float64)[:, None]
    div = np.exp(np.arange(0, D, 2, dtype=np.float64) * (-np.log(10000.0) / D))
    pe = np.zeros((WINDOW, D), dtype=np.float64)
    pe[:, 0::2] = np.sin(pos * div)
    pe[:, 1::2] = np.cos(pos * div)
    return pe.astype(np.float32)


def _block_token_order(blk):
    """Within-core token index for each of a block's u columns, in order.

    AD blocks (k-major): [tile h][k 0..9][agent]  -> tok = a*T + h*10 + k
    B blocks (agent-major): [tile h][agent 70][k] -> tok = a*T + k
    """
    off, n = BLK_OFF[blk], BLK_N[blk]
    if blk < N_AD:
        h = np.arange(2)[:, None, None]
        k = np.arange(10)[None, :, None]
        a = np.arange(n)[None, None, :]
        tok = (off + a) * T + h * 10 + k
    else:
        half = n // 2
        h = np.arange(2)[:, None, None]
        a = np.arange(half)[None, :, None]
        k = np.arange(T)[None, None, :]
        tok = (off + h * half + a) * T + k
    return tok.reshape(-1)


_PERM_LO = np.concatenate([_block_token_order(lo) for lo, hi in SLAB_PAIRS])
_PERM_HI = np.concatenate([_block_token_order(hi) for lo, hi in SLAB_PAIRS])


def _host_inputs(pre_seq, W_in, b_in, pre_timesteps):
    """Per-core u ([117, TOK_C/2] fp16, slab-packed) + shared wf fp16."""
    pe = _build_pe()
    wf = np.concatenate(
        [
            W_in.astype(np.float16),
            b_in.astype(np.float16)[None],
            pe.astype(np.float16),
        ],
        axis=0,
    )  # [53, 256]

    t = pre_timesteps.astype(np.int64)
    oh = np.zeros((WINDOW, N_TOK), dtype=np.float16)
    oh[t, np.arange(N_TOK)] = 1.0
    ones = np.ones((N_TOK,), dtype=np.float16)
    u_full = np.concatenate(
        [
            pre_seq[:, 0].astype(np.float16)[None],
            pre_seq[:, 1].astype(np.float16)[None],
            ones[None],
            oh,
        ],
        axis=0,
    )  # [53, N_TOK] fp16
    wf2 = np.zeros((117, D), dtype=np.float16)
    wf2[0:53] = wf
    wf2[64:117] = wf
    in_maps = []
    for c in range(N_CORES):
        sl = u_full[:, c * TOK_C : (c + 1) * TOK_C]
        u2 = np.zeros((117, TOK_C // 2), dtype=np.float16)
        u2[0:53] = sl[:, _PERM_LO]
        u2[64:117] = sl[:, _PERM_HI]
        in_maps.append({"u": np.ascontiguousarray(u2), "wf": wf2})
    return in_maps


def _mm_splits(width):
    """<=512-col chunks that never cross a 512-fp32 PSUM bank boundary."""
    splits, off = [], 0
    while off < width:
        n = min(512, width - off)
        splits.append((off, n))
        off += n
    return splits


def _build_nc(reps=1, mm_only=False):
    nc = bacc.Bacc(None)
    u = nc.dram_tensor("u", [117, TOK_C // 2], F16, kind="ExternalInput")
    wf = nc.dram_tensor("wf", [117, D], F16, kind="ExternalInput")
    out = nc.dram_tensor("out", [2, 128, AG_C], F32, kind="ExternalOutput")

    slab_off = np.cumsum([0] + SLAB_W).tolist()

    with TileContext(nc) as tc:
        with (
            tc.tile_pool(name="consts", bufs=1) as consts,
            tc.tile_pool(name="uslabs", bufs=2) as uslabs,
            tc.tile_pool(name="outs", bufs=2) as outs,
            tc.tile_pool(name="psum", bufs=2, space="PSUM") as psum_pool,
            tc.tile_pool(name="hbuf", bufs=3) as hbuf_pool,
            tc.tile_pool(name="s10", bufs=3) as s10_pool,
            tc.tile_pool(name="s5", bufs=3) as s5_pool,
            tc.tile_pool(name="t2", bufs=3) as t2_pool,
            tc.tile_pool(name="hm", bufs=4) as hm_pool,
        ):
            wf_sb = consts.tile([117, D], F16)
            nc.sync.dma_start(out=wf_sb[:], in_=wf[:])

            TTMAX = mybir.AluOpType.max
            X = mybir.AxisListType.X

            for rep in range(reps):
                # 4 wide DMAs; slab s holds blocks 2s (rows 0:53) and 2s+1
                # (rows 64:117); per-tag bufs=2 double-buffers across reps
                slabs = []
                for s in range(4):
                    slab = uslabs.tile(
                        [117, SLAB_W[s]], F16, tag=f"slab{s}", bufs=2,
                        name=f"slab{s}",
                    )
                    nc.sync.dma_start(
                        out=slab[:], in_=u[:, slab_off[s] : slab_off[s + 1]]
                    )
                    slabs.append(slab)

                for m in range(2):
                    out_sb = outs.tile([128, AG_C], F32)
                    if mm_only:
                        nc.gpsimd.memset(out_sb[:], 0.0)
                    pend = []  # staged AD blocks awaiting a full triple
                    # B blocks (DVE-only) interleaved between AD triples
                    # (ACT-heavy) so both engines stay busy throughout.
                    for blk in (0, 1, 2, 6, 3, 4, 5, 7):
                        s = blk // 2
                        row0 = 64 if blk % 2 else 0
                        rows = slice(row0, row0 + K_U)
                        tpos = (row0, 0)
                        lhsT = wf_sb[rows, m * 128 : (m + 1) * 128]
                        tw = AD_TILE if blk < N_AD else B_TILE
                        pts = []
                        for h in range(2):
                            pt = psum_pool.tile([128, PSUM_W], F32)
                            for off, n in _mm_splits(tw):
                                o = h * tw + off
                                nc.tensor.matmul(
                                    pt[:, off : off + n],
                                    lhsT,
                                    slabs[s][rows, o : o + n],
                                    tile_position=tpos,
                                )
                            pts.append(pt)
                        if mm_only:
                            continue
                        if blk >= N_AD:
                            # B block: agent-major tiles; one contiguous
                            # tensor_reduce per tile straight to fp32 out
                            for h in range(2):
                                o = BLK_OFF[blk] + h * B_HALF
                                nc.vector.tensor_reduce(
                                    out_sb[:, o : o + B_HALF],
                                    pts[h][:, 0:B_TILE].rearrange(
                                        "p (a k) -> p a k", k=T
                                    ),
                                    axis=X,
                                    op=TTMAX,
                                )
                            continue
                        # AD block: ACT evacuates both tiles into this
                        # triple's interleaved sixths of hb
                        if not pend:
                            hb = hbuf_pool.tile([128, 6 * AD_TILE], F16)
                        else:
                            hb = pend[-1][1]
                        j = len(pend)
                        nc.scalar.copy(
                            hb[:, j * AD_TILE : (j + 1) * AD_TILE],
                            pts[0][:, 0:AD_TILE],
                        )
                        nc.scalar.copy(
                            hb[:, (3 + j) * AD_TILE : (4 + j) * AD_TILE],
                            pts[1][:, 0:AD_TILE],
                        )
                        pend.append((blk, hb))
                        if len(pend) < 3:
                            continue
                        b0 = pend[0][0]
                        pend = []
                        # Stacked 20->1 max tree for blocks (b0, b0+1, b0+2).
                        # hb = [A-t0|B-t0|C-t0|A-t1|B-t1|C-t1], each 1220
                        # cols (10 k-slabs of 122 agents); lanes stay
                        # separate via [p, u, w] views.
                        W3 = 3 * AD_TILE
                        s10 = s10_pool.tile([128, W3], F16)
                        nc.vector.tensor_tensor(
                            s10[:], hb[:, 0:W3], hb[:, W3 : 2 * W3], op=TTMAX
                        )
                        w5 = 5 * AD_AG
                        v10 = s10[:].rearrange("p (u w) -> p u w", u=3)
                        s5 = s5_pool.tile([128, 3 * w5], F16)
                        v5 = s5[:].rearrange("p (u w) -> p u w", u=3)
                        nc.vector.tensor_tensor(
                            v5, v10[:, :, 0:w5], v10[:, :, w5 : 2 * w5],
                            op=TTMAX,
                        )
                        wq = 2 * AD_AG
                        t2 = t2_pool.tile([128, 3 * wq], F16)
                        v2 = t2[:].rearrange("p (u w) -> p u w", u=3)
                        nc.vector.tensor_tensor(
                            v2, v5[:, :, 0:wq], v5[:, :, wq : 2 * wq],
                            op=TTMAX,
                        )
                        t1 = hm_pool.tile([128, 3 * AD_AG], F16)
                        v1 = t1[:].rearrange("p (u w) -> p u w", u=3)
                        nc.vector.tensor_tensor(
                            v1, v2[:, :, 0:AD_AG], v2[:, :, AD_AG : 2 * AD_AG],
                            op=TTMAX,
                        )
                        # Final: max with each unit's 5th k-slab, written to
                        # the triple's contiguous 366-agent span of out_sb.
                        o0 = BLK_OFF[b0]
                        ov = out_sb[:, o0 : o0 + 3 * AD_AG].rearrange(
                            "p (u w) -> p u w", u=3
                        )
                        nc.vector.tensor_tensor(
                            ov, v1[:], v5[:, :, 4 * AD_AG : 5 * AD_AG],
                            op=TTMAX,
                        )
                    nc.sync.dma_start(out=out[m], in_=out_sb[:])

    nc.finalize()
    return nc


_RUNNER = None


def _make_runner():
    """Compile once; return callable(list of per-core input dicts) -> results."""
    import jax
    from jax.sharding import Mesh, PartitionSpec
    from jax.experimental.shard_map import shard_map
    from concourse import bass2jax
    from concourse.bass2jax import _bass_exec_p, partition_id_tensor

    nc = _build_nc()
    bass2jax.install_neuronx_cc_hook()

    partition_name = nc.partition_id_tensor.name if nc.partition_id_tensor else None
    in_names, out_names, out_avals, zero_outs = [], [], [], []
    for alloc in nc.m.functions[0].allocations:
        if not isinstance(alloc, mybir.MemoryLocationSet):
            continue
        name = alloc.memorylocations[0].name
        if alloc.kind == "ExternalInput":
            if name != partition_name:
                in_names.append(name)
        elif alloc.kind == "ExternalOutput":
            out_names.append(name)
            shape = tuple(alloc.tensor_shape)
            dtype = mybir.dt.np(alloc.dtype)
            out_avals.append(jax.core.ShapedArray(shape, dtype))
            zero_outs.append(np.zeros(shape, dtype))
    n_params = len(in_names)
    n_outs = len(out_avals)
    all_in_names = in_names + out_names
    if partition_name is not None:
        all_in_names.append(partition_name)

    def _body(*args):
        operands = list(args)
        if partition_name is not None:
            operands.append(partition_id_tensor())
        outs = _bass_exec_p.bind(
            *operands,
            out_avals=tuple(out_avals),
            in_names=tuple(all_in_names),
            out_names=tuple(out_names),
            lowering_input_output_aliases=(),
            sim_require_finite=True,
            sim_require_nnan=True,
            nc=nc,
        )
        return tuple(outs)

    devices = jax.devices()[:N_CORES]
    mesh = Mesh(np.asarray(devices), ("core",))
    in_specs = (PartitionSpec("core"),) * (n_params + n_outs)
    out_specs = (PartitionSpec("core"),) * n_outs
    donate = tuple(range(n_params, n_params + n_outs))
    sharded = jax.jit(
        shard_map(_body, mesh=mesh, in_specs=in_specs, out_specs=out_specs,
                  check_rep=False),
        donate_argnums=donate,
        keep_unused=True,
    )

    def run(in_maps):
        per_core = [[np.asarray(m[name]) for name in in_names] for m in in_maps]
        concat_in = [
            np.concatenate([per_core[c][i] for c in range(N_CORES)], axis=0)
            for i in range(n_params)
        ]
        concat_zeros = [
            np.zeros((N_CORES * z.shape[0], *z.shape[1:]), z.dtype) for z in zero_outs
        ]
        out_arrs = sharded(*concat_in, *concat_zeros)
        return [
            {
                name: np.asarray(out_arrs[i]).reshape(N_CORES, *out_avals[i].shape)[c]
                for i, name in enumerate(out_names)
            }
            for c in range(N_CORES)
        ]

    return run


def _get_runner():
    global _RUNNER
    if _RUNNER is None:
        _RUNNER = _make_runner()
    return _RUNNER


def _make_timed(nc, in_maps_fn):
    """Zero-host-transfer callable for steady-state timing (no donation)."""
    import jax
    from jax.sharding import Mesh, PartitionSpec, NamedSharding
    from jax.experimental.shard_map import shard_map
    from concourse import bass2jax
    from concourse.bass2jax import _bass_exec_p, partition_id_tensor

    bass2jax.install_neuronx_cc_hook()
    partition_name = nc.partition_id_tensor.name if nc.partition_id_tensor else None
    in_names, out_names, out_avals = [], [], []
    for alloc in nc.m.functions[0].allocations:
        if not isinstance(alloc, mybir.MemoryLocationSet):
            continue
        name = alloc.memorylocations[0].name
        if alloc.kind == "ExternalInput":
            if name != partition_name:
                in_names.append(name)
        elif alloc.kind == "ExternalOutput":
            out_names.append(name)
            out_avals.append(
                jax.core.ShapedArray(tuple(alloc.tensor_shape), mybir.dt.np(alloc.dtype))
            )
    n_params = len(in_names)
    all_in_names = in_names + out_names + ([partition_name] if partition_name else [])

    def _body(*args):
        operands = list(args)
        if partition_name is not None:
            operands.append(partition_id_tensor())
        outs = _bass_exec_p.bind(
            *operands,
            out_avals=tuple(out_avals),
            in_names=tuple(all_in_names),
            out_names=tuple(out_names),
            lowering_input_output_aliases=(),
            sim_require_finite=True,
            sim_require_nnan=True,
            nc=nc,
        )
        return tuple(outs)

    devices = jax.devices()[:N_CORES]
    mesh = Mesh(np.asarray(devices), ("core",))
    nout = len(out_names)
    sharded = jax.jit(
        shard_map(
            _body,
            mesh=mesh,
            in_specs=(PartitionSpec("core"),) * (n_params + nout),
            out_specs=(PartitionSpec("core"),) * nout,
            check_rep=False,
        ),
        keep_unused=True,
    )
    sh = NamedSharding(mesh, PartitionSpec("core"))
    in_maps = in_maps_fn()
    per_core = [[np.asarray(m[name]) for name in in_names] for m in in_maps]
    dev_in = [
        jax.device_put(
            np.concatenate([per_core[c][i] for c in range(N_CORES)], axis=0), sh
        )
        for i in range(n_params)
    ]
    dev_zero = [
        jax.device_put(np.zeros((N_CORES * a.shape[0], *a.shape[1:]), a.dtype), sh)
        for a in out_avals
    ]

    def run():
        return sharded(*dev_in, *dev_zero)

    return run


def _get_timed_callable(inputs, reps=1, mm_only=False):
    nc = _build_nc(reps=reps, mm_only=mm_only)
    return _make_timed(
        nc,
        lambda: _host_inputs(
            inputs["pre_seq"], inputs["W_in"], inputs["b_in"], inputs["pre_timesteps"]
        ),
    )


def kernel(pre_seq, W_in, b_in, pre_timesteps, pre_agents, n_agents):
    run = _get_runner()
    in_maps = _host_inputs(pre_seq, W_in, b_in, pre_timesteps)
    results = run(in_maps)
    out = np.empty((N_AGENTS, D), dtype=np.float32)
    for c in range(N_CORES):
        o = results[c]["out"]  # [2, 128, AG_C]
        out[c * AG_C : (c + 1) * AG_C] = o.transpose(2, 0, 1).reshape(AG_C, D)
    return out
